# revision 19
# baseline (speedup 1.0000x reference)
"""Guide-token attention kernel for Trainium2 (8 NeuronCores).

Module: y[b] = softmax(((Q+tQ) @ (K+tK)^T)/sqrt(hd)) @ V  per head, where
  Q = x @ Wq^T + bq, K = x @ Wk^T + bk, V = x @ Wv^T + bv,
  tQ/tK are projections of a per-batch guide token (broadcast over seq).

Shapes: x [4, 1024, 1024], tokens [4, 1, 1024], W* [1024, 1024], b* [1024].
H=16 heads, hd=64.

Sharding: 8 cores = 4 batches x 2 head-groups (8 heads each); weights
column-sharded per head group; each core sees one batch -> no cross-core
communication.

Layout (PE contracts over the partition axis; no on-chip transposes):
  - host pre-transposes x[b] -> xT [D, S] and W slices -> wT [D, 512] (bf16),
    and precomputes the tiny guide-token adds (tq + 2*bq etc.).
  - QT/KT computed transposed [feat, S]; V computed natural [S, feat].
  - scores computed directly transposed per head: sT[k, q] = cK @ cQ^T
    (lhsT = cKT slice, rhs = cQT slice, contraction = hd = 64).
  - exp on ScalarE over two-bank PSUM tiles [128, 2, 512] (amortizes the
    ~352-cycle ACTIVATE overhead), writing bf16 probs. Softmax max-
    subtraction skipped: |scores| <= ~15 so exp is safely in fp32/bf16 range.
  - AV: lhsT = V chunk [k, 64] + appended ones column (row 64 of the output
    accumulates the softmax denominator), rhs = probsT [k, q], accumulated
    over k chunks -> [65, q] PSUM.
  - normalize: denominator row -> SBUF, reciprocal (fast-approx), GpSimd
    partition_broadcast to 64 rows, one VectorE multiply -> yT [feat, S].
  - host reassembles y[b][:, cols] = yT^T and adds bv once at the end
    (softmax rows sum to 1, so y = softmax@V0 + bv exactly).

Schedule (HAM-aware): the PE instruction stream is kept dense so the clock
gate stays at 8/8. QK(ft0) runs first; the h0-h3 attention units then
interleave V and QK(ft1) matmuls as fillers between score pairs (3 filler
MMs per pair exactly consumes the 96 remaining projection MMs at the rate
ACT drains exps); the h4-h7 score units interleave with the h0-h3 AV units.
"""

import os

import numpy as np
import ml_dtypes

import concourse.bass as bass
import concourse.tile as tile
from concourse import bacc
from concourse import mybir
from concourse.bass_utils import run_bass_kernel_spmd

B = 4
S = 1024
D = 1024
H = 16
HD = 64
NCORES = 8
FPG = 512          # features per head-group (8 heads * 64)
NKC = D // 128     # contraction chunks for projections
NFT = FPG // 128   # feature tiles per group
NST = S // 128     # sequence tiles
NQB = S // 512     # 512-wide query blocks
HPG = 8            # heads per group
NPAIR = NST // 2   # kt pairs per unit

BF16 = mybir.dt.bfloat16
F32 = mybir.dt.float32

_CACHE = {}


def _build():
    nc = bacc.Bacc()

    # Inputs pre-shuffled on host to [128 partitions, kc, cols] so HBM order
    # matches SBUF order: per-partition-contiguous 16KB/8KB DMA packets
    # instead of 2KB (DMA is packet-bound otherwise).
    xT = nc.declare_dram_parameter("xT", [128, NKC, S], BF16, isOutput=False)
    wqT = nc.declare_dram_parameter("wqT", [128, NKC, FPG], BF16, isOutput=False)
    wkT = nc.declare_dram_parameter("wkT", [128, NKC, FPG], BF16, isOutput=False)
    wvT = nc.declare_dram_parameter("wvT", [128, NKC, FPG], BF16, isOutput=False)
    qadd = nc.declare_dram_parameter("qadd", [128, NFT], F32, isOutput=False)
    kadd = nc.declare_dram_parameter("kadd", [128, NFT], F32, isOutput=False)
    yT = nc.declare_dram_parameter("yT", [FPG, S], F32, isOutput=True)

    with tile.TileContext(nc) as tc:
        with (
            tc.tile_pool(name="persist", bufs=1) as persist,
            tc.tile_pool(name="probs", bufs=44) as probs_pool,
            tc.tile_pool(name="norm", bufs=4) as norm_pool,
            tc.tile_pool(name="psP", bufs=2, space=bass.MemorySpace.PSUM) as psP,
            tc.tile_pool(name="psA", bufs=2, space=bass.MemorySpace.PSUM) as psA,
            tc.tile_pool(name="psAV", bufs=2, space=bass.MemorySpace.PSUM) as psAV,
        ):
            # ---- persistent SBUF tensors ----
            xt = persist.tile([128, NKC, S], BF16)
            wq = persist.tile([128, NKC, FPG], BF16)
            wk = persist.tile([128, NKC, FPG], BF16)
            wv = persist.tile([128, NKC, FPG], BF16)
            qa = persist.tile([128, NFT], F32)
            ka = persist.tile([128, NFT], F32)
            cq = persist.tile([128, NFT, S], BF16)          # cQT/8  [feat, S]
            ck = persist.tile([128, NFT, S], BF16)          # cKT    [feat, S]
            vt = persist.tile([128, NST, HPG, HD + 1], BF16)  # V' + ones col
            yt = persist.tile([128, NFT, S], F32)           # yT [feat, S]

            # ---- input DMAs (wq/x first: QK ft0 starts the kernel) ----
            # Partition-sliced so transfers parallelize across DMA queues
            # while keeping 16KB per-partition-contiguous packets.
            nc.sync.dma_start(out=qa[:], in_=qadd[:])
            nc.sync.dma_start(out=ka[:], in_=kadd[:])
            for dst, srcp in ((wq, wqT), (xt, xT), (wk, wkT), (wv, wvT)):
                for p in range(4):
                    ps = slice(p * 32, (p + 1) * 32)
                    nc.sync.dma_start(out=dst[ps, :, :], in_=srcp[ps, :, :])

            nc.vector.memset(vt[:, :, :, HD:HD + 1], 1.0)

            # ---- HAM pre-warm: dummy matmuls while input DMAs stream ----
            # The PE clock gate needs ~3.4us of sustained activity to go
            # 8/8; burn the DMA head (~10us) on throwaway matmuls so the
            # real projections start at full clock.
            wrm = persist.tile([128, 512], BF16)
            nc.gpsimd.memset(wrm[:], 0.0)
            wacc = psAV.tile([128, 512], F32, tag="psAV")
            for _ in range(12):
                nc.tensor.matmul(
                    wacc[:], wrm[:, 0:128], wrm[:], start=True, stop=True
                )

            # ---- projection building blocks ----
            def qk_group(which, ft, sb):
                """QT/KT [feat tile, S block] accumulated over D chunks,
                evicted to bf16 with the guide-token add (+1/8 scale for Q)."""
                w_sb, add_sb, scale, dst = (
                    (wq, qa, 0.125, cq) if which == "q" else (wk, ka, 1.0, ck)
                )
                acc = psP.tile([128, 512], F32, tag="psP")
                for kc in range(NKC):
                    yield lambda kc=kc, acc=acc: nc.tensor.matmul(
                        acc[:],
                        w_sb[:, kc, ft * 128:(ft + 1) * 128],
                        xt[:, kc, sb * 512:(sb + 1) * 512],
                        start=(kc == 0),
                        stop=(kc == NKC - 1),
                    )
                yield lambda acc=acc: nc.vector.tensor_scalar(
                    out=dst[:, ft, sb * 512:(sb + 1) * 512],
                    in0=acc[:],
                    scalar1=scale,
                    scalar2=add_sb[:, ft:ft + 1],
                    op0=mybir.AluOpType.mult,
                    op1=mybir.AluOpType.add,
                )

            def v_group(st):
                """V [S tile, feat] natural layout, strided into vt."""
                acc = psP.tile([128, 512], F32, tag="psP")
                for kc in range(NKC):
                    yield lambda kc=kc, acc=acc: nc.tensor.matmul(
                        acc[:],
                        xt[:, kc, st * 128:(st + 1) * 128],
                        wv[:, kc, :],
                        start=(kc == 0),
                        stop=(kc == NKC - 1),
                    )
                yield lambda acc=acc: nc.vector.tensor_copy(
                    out=vt[:, st, :, 0:HD], in_=acc[:]
                )

            def run(gen):
                for op in gen:
                    op()

            # filler stream: QK ft2/ft3 + all of V (96 MMs + evictions)
            def filler_stream():
                for which in ("q", "k"):
                    for ft in (2, 3):
                        for sb in range(NQB):
                            yield from qk_group(which, ft, sb)
                for st in range(NST):
                    yield from v_group(st)

            # ---- attention building blocks ----
            def unit_scores(hp, qb, filler=None):
                """Score MMs for head pair (2hp, 2hp+1), one 512-wide query
                block. The two heads' operands live on partitions 0-63 /
                64-127 -> different PE row groups, so their K=64 matmuls run
                concurrently (row tiling). exp pairs on ScalarE -> bf16
                probs. Pulls filler ops to keep the PE stream dense."""
                ft = hp
                qsl = slice(qb * 512, (qb + 1) * 512)
                pairsA, pairsB = [], []
                for p in range(NPAIR):
                    scA = psA.tile([128, 2, 512], F32, tag="psA")
                    scB = psA.tile([128, 2, 512], F32, tag="psA")
                    for j in range(2):
                        kt = 2 * p + j
                        ksl = slice(kt * 128, (kt + 1) * 128)
                        nc.tensor.matmul(
                            scA[:, j, :], ck[0:64, ft, ksl], cq[0:64, ft, qsl],
                            start=True, stop=True,
                        )
                        nc.tensor.matmul(
                            scB[:, j, :], ck[64:128, ft, ksl], cq[64:128, ft, qsl],
                            start=True, stop=True,
                        )
                    prA = probs_pool.tile([128, 2, 512], BF16, tag="probs")
                    nc.scalar.activation(
                        out=prA[:], in_=scA[:],
                        func=mybir.ActivationFunctionType.Exp,
                    )
                    prB = probs_pool.tile([128, 2, 512], BF16, tag="probs")
                    nc.scalar.activation(
                        out=prB[:], in_=scB[:],
                        func=mybir.ActivationFunctionType.Exp,
                    )
                    pairsA.append(prA)
                    pairsB.append(prB)
                    if filler is not None:
                        for _ in range(8):
                            op = next(filler, None)
                            if op is not None:
                                op()
                return pairsA, pairsB

            def head_av(h, qb, pairs):
                """AV accumulation + softmax normalization -> yt slice."""
                pbase = (h % 2) * 64
                ft = h // 2
                qsl = slice(qb * 512, (qb + 1) * 512)
                av = psAV.tile([HD + 1, 512], F32, tag="psAV")
                for kt in range(NST):
                    nc.tensor.matmul(
                        av[:],
                        vt[:, kt, h, :],
                        pairs[kt // 2][:, kt % 2, :],
                        start=(kt == 0),
                        stop=(kt == NST - 1),
                    )
                den = norm_pool.tile([1, 512], F32, tag="den")
                nc.vector.tensor_copy(out=den[:], in_=av[HD:HD + 1, :])
                rec = norm_pool.tile([1, 512], F32, tag="rec")
                nc.vector.reciprocal_approx_fast(out=rec[:], in_=den[:])
                recb = norm_pool.tile([HD, 512], F32, tag="recb")
                nc.gpsimd.partition_broadcast(recb[:], rec[:])
                nc.vector.tensor_tensor(
                    out=yt[pbase:pbase + 64, ft, qsl],
                    in0=av[0:HD, :],
                    in1=recb[:],
                    op=mybir.AluOpType.mult,
                )

            def unit_av(hp, qb, pr):
                head_av(2 * hp, qb, pr[0])
                head_av(2 * hp + 1, qb, pr[1])

            # ---- schedule ----
            # Phase 1: QK ft0/ft1 dense (head pairs 0-1 depend only on these).
            for which in ("q", "k"):
                for ft in (0, 1):
                    for sb in range(NQB):
                        run(qk_group(which, ft, sb))

            units = [(hp, qb) for hp in range(HPG // 2) for qb in range(NQB)]
            early, late = units[:4], units[4:]

            # Phase 2: early score units with projection fillers.
            filler = filler_stream()
            pairs_of = {}
            for hp, qb in early:
                pairs_of[(hp, qb)] = unit_scores(hp, qb, filler=filler)
            for op in filler:   # drain any remainder (V must precede AV)
                op()

            # Output DMA per feature tile as soon as both its units are done.
            done_units = set()

            def maybe_flush(hp, qb):
                done_units.add((hp, qb))
                if all((hp, q) in done_units for q in range(NQB)):
                    nc.sync.dma_start(
                        out=yT[hp * 128:(hp + 1) * 128, :], in_=yt[:, hp, :]
                    )

            # Phase 3: early AV interleaved with late score units.
            for i, (hp, qb) in enumerate(late):
                unit_av(*early[i], pairs_of.pop(early[i]))
                maybe_flush(*early[i])
                pairs_of[(hp, qb)] = unit_scores(hp, qb)

            # Phase 4: late AV units.
            for hp, qb in late:
                unit_av(hp, qb, pairs_of.pop((hp, qb)))
                maybe_flush(hp, qb)

    nc.finalize()
    return nc


def _get_nc():
    if "nc" not in _CACHE:
        _CACHE["nc"] = _build()
    return _CACHE["nc"]


def kernel(x, tokens, Wq, bq, Wk, bk, Wv, bv):
    x = np.asarray(x, dtype=np.float32)
    tokens = np.asarray(tokens, dtype=np.float32)
    Wq = np.asarray(Wq, dtype=np.float32)
    Wk = np.asarray(Wk, dtype=np.float32)
    Wv = np.asarray(Wv, dtype=np.float32)
    bq = np.asarray(bq, dtype=np.float32)
    bk = np.asarray(bk, dtype=np.float32)
    bv = np.asarray(bv, dtype=np.float32)

    bf16 = ml_dtypes.bfloat16
    in_maps = []
    for c in range(NCORES):
        b, g = divmod(c, 2)
        rows = slice(g * FPG, (g + 1) * FPG)
        tq = tokens[b, 0] @ Wq[rows].T + 2.0 * bq[rows]   # [512]
        tk = tokens[b, 0] @ Wk[rows].T + 2.0 * bk[rows]
        def pack(aT):
            # [D, C] -> [128, NKC, C]: partition-major to match SBUF layout
            return np.ascontiguousarray(
                aT.reshape(NKC, 128, aT.shape[1]).transpose(1, 0, 2)
            ).astype(bf16)

        in_maps.append({
            "xT": pack(x[b].T),
            "wqT": pack(Wq[rows].T),
            "wkT": pack(Wk[rows].T),
            "wvT": pack(Wv[rows].T),
            "qadd": np.ascontiguousarray((tq / 8.0).reshape(NFT, 128).T).astype(np.float32),
            "kadd": np.ascontiguousarray(tk.reshape(NFT, 128).T).astype(np.float32),
        })

    nc = _get_nc()
    trace = bool(int(os.environ.get("KERNEL_TRACE", "0")))
    res = run_bass_kernel_spmd(nc, in_maps, core_ids=list(range(NCORES)), trace=trace)
    if trace:
        _CACHE["last_results"] = res

    y = np.empty((B, S, D), dtype=np.float32)
    for c in range(NCORES):
        b, g = divmod(c, 2)
        y[b, :, g * FPG:(g + 1) * FPG] = res.results[c]["yT"].T
    y += bv[None, None, :]
    return y
